# revision 6
# baseline (speedup 1.0000x reference)
"""GPT-2 style multi-head attention on 8 Trainium2 cores (Bass/Tile).

Problem: B=2, T=2048, C=1024, H=16 heads, D=64, fp32.

Sharding (hardcoded): 2 groups x 4 cores; group g handles batch b=g.
Within a group, rank r computes heads [4r, 4r+4) (tensor parallel over
heads: c_attn column slices), then AllGather of y^T across the group,
then each core computes a 256-column slice of the output projection
(c_proj column slice) plus bias.

Kernel dataflow per core (all matmuls in float32r, full PE rate at N>=256):
  stage 1: x -> (PE transpose) x^T;  qk^T = W_qk^T @ x^T (lhsT = W_qk,
           natural layout, bias per-partition);  V = x @ W_v (lhsT = x^T,
           bias via ones-row matmul).  V is stored per k-tile with an
           appended ones column so the AV matmul also produces the
           softmax denominator for free.
  stage 2: per (head, 512-wide q block): scores^T[k,q] = K^T.T @ Q^T in
           PSUM -> exp(0.125*s) on ACT -> causal mask multiply (diagonal
           tiles only) -> y_aug^T[65,512] += V_aug^T @ exp^T accumulated
           over k tiles (row 64 = sum of exp = softmax denominator).
           Normalize: reciprocal of row 64, broadcast over 64 partitions
           with a ones matmul, multiply.
  stage 3: AllGather y^T (per q-block chunks, overlapped with attention),
           out[:, 256-col slice] = y^T.T @ W_p slice + bias.
"""

import numpy as np

import concourse.bass as bass
import concourse.mybir as mybir
import concourse.tile as tile
from concourse import bacc

P = 128
B, T_FULL, C, H, D = 2, 2048, 1024, 16, 64
F32 = mybir.dt.float32
F32R = mybir.dt.float32r
EXP = mybir.ActivationFunctionType.Exp
ADD = mybir.AluOpType.add
MUL = mybir.AluOpType.mult
BYPASS = mybir.AluOpType.bypass


def _r(ap):
    """Matmul operands are allocated as float32r already; identity."""
    return ap


class Cfg:
    def __init__(self, n_cores, group_size, T):
        self.n_cores = n_cores
        self.GS = group_size               # cores per batch group
        self.T = T                         # sequence length handled per core
        self.HL = H // group_size          # heads per core
        assert self.HL % 2 == 0
        self.NP = C // group_size          # output-projection columns per core
        self.CC = C // P                   # contraction chunks (8)
        self.TB = T // 512                 # stage-1 t-blocks
        self.QB = T // 512                 # q blocks
        self.KT = T // P                   # k tiles
        self.QKCH = self.HL                # qk^T partition chunks (Q:HL/2, K:HL/2)
        self.VW = 68                       # per-head V stride: 64 V + 1 ones + pad
        if n_cores == 8:
            self.replica_groups = [[0, 1, 2, 3], [4, 5, 6, 7]]
        elif n_cores == 4:
            self.replica_groups = [[0, 1], [2, 3]]
        elif n_cores == 1:
            self.replica_groups = [[0]]
        else:
            raise ValueError(n_cores)


CFG_FULL = Cfg(8, 4, T_FULL)


def emit(tc, outs, ins, cfg):
    """Emit the SPMD program. outs/ins are dicts of DRAM APs."""
    nc = tc.nc
    GS, T, HL, NP, CC, VW = cfg.GS, cfg.T, cfg.HL, cfg.NP, cfg.CC, cfg.VW
    QKCH = cfg.QKCH

    x = ins["x"]              # [T, C]
    wqk = ins["wqk"]          # [C, HL*128]  (Q cols | K cols)
    wv = ins["wv"]            # [C, HL*64]
    bqk = ins["bqk"]          # [P, HL]  (chunk-major per-partition bias)
    bv = ins["bv"]            # [1, HL*64]
    wp = ins["wp"]            # [C, NP]
    bp = ins["bp"]            # [1, NP]
    masks = ins["masks"]      # [P, 4, 512]
    ident = ins["ident"]      # [P, P]
    out = outs["out"]         # [T, NP]

    with (
        tc.tile_pool(name="persist", bufs=1) as persist,
        tc.tile_pool(name="dram", bufs=1, space="DRAM") as dram,
    ):
        # ---- persistent SBUF tensors ----
        qkT = persist.tile([P, QKCH, T], F32R, tag="qkT")
        vsb = persist.tile([P, cfg.KT, HL * VW], F32R, tag="vsb")
        mask_sb = persist.tile([P, 4, 512], F32R, tag="mask")
        ident_sb = persist.tile([P, P], F32, tag="ident")
        ones_row = persist.tile([1, P], F32R, tag="ones_row")
        ones65 = persist.tile([65, 64], F32R, tag="ones65")
        wp_sb = persist.tile([P, CC, NP], F32R, tag="wp")
        bp_sb = persist.tile([1, NP], F32R, tag="bp")

        nc.sync.dma_start(mask_sb[:], masks.bitcast(F32R))
        nc.sync.dma_start(ident_sb[:], ident)
        nc.sync.dma_start(wp_sb[:], wp.rearrange("(c p) n -> p c n", p=P).bitcast(F32R))
        nc.sync.dma_start(bp_sb[:], bp.bitcast(F32R))
        # memset can't write float32r; memset f32 scratch, copy-convert over.
        scratch1 = persist.tile([P, max(P, cfg.KT * HL)], F32, tag="scratch1")
        nc.vector.memset(scratch1[:], 1.0)
        nc.vector.tensor_copy(ones_row[:], scratch1[0:1, 0:P])
        nc.vector.tensor_copy(ones65[64:65, :], scratch1[64:65, 0:64])
        # ones columns inside the V tile (col 64 of each head's 68-wide slot)
        vsb_h = vsb.rearrange("p k (h w) -> p k h w", w=VW)
        nc.vector.tensor_copy(
            vsb_h[:, :, :, 64:65],
            scratch1[:, 0:cfg.KT * HL].rearrange(
                "p (k h o) -> p k h o", k=cfg.KT, h=HL, o=1),
        )

        # ---- stage 1: x^T, qk^T, V ----
        with (
            tc.tile_pool(name="s1", bufs=2) as s1,
            tc.tile_pool(name="s1w", bufs=1) as s1w,
            tc.tile_pool(name="ps_tp", bufs=2, space="PSUM") as ps_tp,
            tc.tile_pool(name="ps_qk", bufs=2, space="PSUM") as ps_qk,
            tc.tile_pool(name="ps_v", bufs=2, space="PSUM") as ps_v,
        ):
            wqk_sb = s1w.tile([P, CC, QKCH * P], F32R, tag="wqk")
            wv_sb = s1w.tile([P, CC, HL * D], F32R, tag="wv")
            bqk_sb = s1w.tile([P, QKCH], F32, tag="bqk")
            bv_sb = s1w.tile([1, HL * D], F32R, tag="bv")
            nc.sync.dma_start(wqk_sb[:], wqk.rearrange("(c p) m -> p c m", p=P).bitcast(F32R))
            nc.sync.dma_start(wv_sb[:], wv.rearrange("(c p) m -> p c m", p=P).bitcast(F32R))
            nc.sync.dma_start(bqk_sb[:], bqk)
            nc.sync.dma_start(bv_sb[:], bv.bitcast(F32R))

            x_r = x.rearrange("(a s p) c -> a p s c", p=P, s=4)
            for tb in range(cfg.TB):
                x_sb = s1.tile([P, 4, C], F32, tag="x")
                nc.sync.dma_start(x_sb[:], x_r[tb])
                xT = s1.tile([P, CC, 512], F32R, tag="xT")
                for ts in range(4):
                    for cc in range(CC):
                        tp = ps_tp.tile([P, P], F32, tag="tp")
                        nc.tensor.transpose(
                            tp[:], x_sb[:, ts, cc * P:(cc + 1) * P], ident_sb[:]
                        )
                        nc.vector.tensor_copy(
                            xT[:, cc, ts * P:(ts + 1) * P], tp[:]
                        )
                # qk^T: lhsT = W chunk, rhs = x^T chunk
                for m in range(QKCH):
                    acc = ps_qk.tile([P, 512], F32, tag="qk")
                    for cc in range(CC):
                        nc.tensor.matmul(
                            acc[:],
                            _r(wqk_sb[:, cc, m * P:(m + 1) * P]),
                            _r(xT[:, cc, :]),
                            start=(cc == 0),
                            stop=(cc == CC - 1),
                        )
                    nc.vector.tensor_scalar_add(
                        qkT[:, m, tb * 512:(tb + 1) * 512], acc[:],
                        bqk_sb[:, m:m + 1],
                    )
                # V natural: lhsT = x^T chunk, rhs = W_v
                for ts in range(4):
                    kt = tb * 4 + ts
                    vp = ps_v.tile([P, HL * D], F32, tag="v")
                    for cc in range(CC):
                        nc.tensor.matmul(
                            vp[:],
                            _r(xT[:, cc, ts * P:(ts + 1) * P]),
                            _r(wv_sb[:, cc, :]),
                            start=(cc == 0),
                            stop=False,
                        )
                    nc.tensor.matmul(
                        vp[:], _r(ones_row[:1, :]), _r(bv_sb[:1, :]),
                        start=False, stop=True,
                    )
                    nc.vector.tensor_copy(
                        vsb_h[:, kt, :, 0:64],
                        vp.rearrange("p (h d) -> p h d", d=D),
                    )

        # ---- stages 2+3 interleaved ----
        with (
            tc.tile_pool(name="s2", bufs=4) as s2,
            tc.tile_pool(name="s3", bufs=2) as s3,
            tc.tile_pool(name="ps_s", bufs=2, space="PSUM") as ps_s,
            tc.tile_pool(name="ps_y", bufs=2, space="PSUM") as ps_y,
            tc.tile_pool(name="ps_bc", bufs=1, space="PSUM") as ps_bc,
            tc.tile_pool(name="ps_o", bufs=2, space="PSUM") as ps_o,
        ):
            ag_in = [
                dram.tile([HL * D, 512], F32, tag=f"agin{qb}", name=f"agin{qb}")
                for qb in range(cfg.QB)
            ]
            ag_out = [
                dram.tile([GS * HL * D, 512], F32, tag=f"agout{qb}",
                          name=f"agout{qb}")
                for qb in range(cfg.QB)
            ]

            def attention(qb):
                for h in range(HL):
                    pb = slice((h % 2) * 64, (h % 2) * 64 + 64)
                    qch, kch = h // 2, QKCH // 2 + h // 2
                    q_ap = qkT[pb, qch, qb * 512:(qb + 1) * 512]
                    y = ps_y.tile([65, 512], F32, tag="y")
                    nkt = 4 * qb + 4
                    for kt in range(nkt):
                        s = ps_s.tile([P, 512], F32, tag="s")
                        nc.tensor.matmul(
                            s[:], _r(qkT[pb, kch, kt * P:(kt + 1) * P]),
                            _r(q_ap), start=True, stop=True,
                        )
                        e = s2.tile([P, 512], F32R, tag="e")
                        nc.scalar.activation(e[:], s[:], EXP, scale=0.125)
                        j = kt - 4 * qb
                        if j >= 0:
                            nc.vector.tensor_mul(e[:], e[:], mask_sb[:, j, :])
                        nc.tensor.matmul(
                            y[:], _r(vsb[:, kt, h * VW:h * VW + 65]), _r(e[:]),
                            start=(kt == 0), stop=(kt == nkt - 1),
                        )
                    rec = s2.tile([65, 512], F32R, tag="rec")
                    with nc.allow_low_precision(
                        reason="float32r reciprocal feeds a broadcast matmul;"
                        " ~1e-6 relative is plenty for softmax denominators"
                    ):
                        nc.vector.reciprocal(rec[64:65, :], y[64:65, :])
                    bc = ps_bc.tile([64, 512], F32, tag="bc")
                    nc.tensor.matmul(
                        bc[:], _r(ones65[64:65, :]), _r(rec[64:65, :]),
                        start=True, stop=True,
                    )
                    bc_sb = s2.tile([64, 512], F32, tag="bc_sb")
                    nc.vector.tensor_copy(bc_sb[:], bc[:])
                    yn = s2.tile([64, 512], F32, tag="yn")
                    nc.vector.tensor_mul(yn[:], y[0:64, :], bc_sb[:])
                    nc.sync.dma_start(ag_in[qb][h * 64:(h + 1) * 64, :], yn[:])

            def allgather(qb):
                nc.gpsimd.collective_compute(
                    "AllGather", BYPASS,
                    replica_groups=cfg.replica_groups,
                    ins=[ag_in[qb].opt()],
                    outs=[ag_out[qb].opt()],
                )

            def proj(qb):
                ag_sb = s3.tile([P, CC, 512], F32R, tag="ag")
                nc.sync.dma_start(
                    ag_sb[:],
                    ag_out[qb].rearrange("(c p) t -> p c t", p=P).bitcast(F32R),
                )
                for tt in range(4):
                    op = ps_o.tile([P, NP], F32, tag="o")
                    for cc in range(CC):
                        nc.tensor.matmul(
                            op[:], _r(ag_sb[:, cc, tt * P:(tt + 1) * P]),
                            _r(wp_sb[:, cc, :]), start=(cc == 0), stop=False,
                        )
                    nc.tensor.matmul(
                        op[:], _r(ones_row[:1, :]), _r(bp_sb[:1, :]),
                        start=False, stop=True,
                    )
                    o_sb = s3.tile([P, NP], F32, tag="osb")
                    nc.vector.tensor_copy(o_sb[:], op[:])
                    row = (qb * 4 + tt) * P
                    nc.sync.dma_start(out[row:row + P, :], o_sb[:])

            # interleave: attention(qb) then AG(qb); proj(qb) after AG(qb),
            # emitted after attention(qb+1) so the collective overlaps compute.
            for qb in range(cfg.QB):
                attention(qb)
                allgather(qb)
                if qb > 0:
                    proj(qb - 1)
            proj(cfg.QB - 1)


def make_core_inputs(x_full, c_attn_w, c_attn_b, c_proj_w, c_proj_b, cfg, core):
    """Host-side input sharding for one core."""
    GS, HL, NP, T = cfg.GS, cfg.HL, cfg.NP, cfg.T
    g, rk = divmod(core, GS)
    g = g % B  # tolerate more groups than batches (sim configs)
    hs = slice(rk * HL * D, (rk + 1) * HL * D)
    wq = c_attn_w[:, 0 * C:1 * C][:, hs]
    wk = c_attn_w[:, 1 * C:2 * C][:, hs]
    wv = c_attn_w[:, 2 * C:3 * C][:, hs]
    bq = c_attn_b[0 * C:1 * C][hs]
    bk = c_attn_b[1 * C:2 * C][hs]
    bv = c_attn_b[2 * C:3 * C][hs]
    cs = slice(rk * NP, (rk + 1) * NP)

    pp = np.arange(P)[:, None, None]
    jj = np.arange(4)[None, :, None]
    qq = np.arange(512)[None, None, :]
    masks = (qq >= pp + 128 * jj).astype(np.float32)

    return {
        "x": np.ascontiguousarray(x_full[g, :T], np.float32),
        "wqk": np.ascontiguousarray(
            np.concatenate([wq, wk], axis=1), np.float32),
        "wv": np.ascontiguousarray(wv, np.float32),
        "bqk": np.ascontiguousarray(
            np.concatenate([bq, bk]).reshape(cfg.QKCH, P).T, np.float32),
        "bv": np.ascontiguousarray(bv[None, :], np.float32),
        "wp": np.ascontiguousarray(c_proj_w[:, cs], np.float32),
        "bp": np.ascontiguousarray(c_proj_b[cs][None, :], np.float32),
        "masks": masks,
        "ident": np.eye(P, dtype=np.float32),
    }


_CACHE = {}


def _build_full():
    if "nc" in _CACHE:
        return _CACHE["nc"]
    cfg = CFG_FULL
    nc = bacc.Bacc(
        "TRN2", target_bir_lowering=False, debug=False,
        num_devices=cfg.n_cores,
    )
    ins = {}
    shapes = {
        "x": (cfg.T, C),
        "wqk": (C, cfg.QKCH * P),
        "wv": (C, cfg.HL * D),
        "bqk": (P, cfg.QKCH),
        "bv": (1, cfg.HL * D),
        "wp": (C, cfg.NP),
        "bp": (1, cfg.NP),
        "masks": (P, 4, 512),
        "ident": (P, P),
    }
    for name, shape in shapes.items():
        ins[name] = nc.dram_tensor(
            name, list(shape), F32, kind="ExternalInput").ap()
    outs = {
        "out": nc.dram_tensor(
            "out", [cfg.T, cfg.NP], F32, kind="ExternalOutput").ap()
    }
    with tile.TileContext(nc) as tc:
        emit(tc, outs, ins, cfg)
    nc.compile()
    _CACHE["nc"] = nc
    return nc


def kernel(**inputs):
    from concourse.bass_utils import run_bass_kernel_spmd

    cfg = CFG_FULL
    x = np.asarray(inputs["x"], np.float32)
    c_attn_w = np.asarray(inputs["c_attn_w"], np.float32)
    c_attn_b = np.asarray(inputs["c_attn_b"], np.float32)
    c_proj_w = np.asarray(inputs["c_proj_w"], np.float32)
    c_proj_b = np.asarray(inputs["c_proj_b"], np.float32)

    nc = _build_full()
    in_maps = [
        make_core_inputs(x, c_attn_w, c_attn_b, c_proj_w, c_proj_b, cfg, core)
        for core in range(cfg.n_cores)
    ]
    res = run_bass_kernel_spmd(nc, in_maps, core_ids=list(range(cfg.n_cores)))
    out = np.empty((B, T_FULL, C), np.float32)
    for core in range(cfg.n_cores):
        g, rk = divmod(core, cfg.GS)
        out[g, :, rk * cfg.NP:(rk + 1) * cfg.NP] = res.results[core]["out"]
    return out
